# revision 34
# baseline (speedup 1.0000x reference)
"""AxialAttention (axis=height) Trainium2 Bass kernel, v2.

Problem: x [B=2,T=4,C=256,H=128,W=128] f32. Lines run along H; each core
owns one (b,t): 128 lines of length L=128, C=256 channels, 8 heads x 32.

Key structure (engine-balanced, fp8 DoubleRow heavy):
  - x streamed per w-block (16 lines) as fp8 hi+lo pair, host pre-laid-out
    so every DMA is 4KB-contiguous per partition.
  - Stage A: q,k = x_hi @ (16*Wqk) via fp8 DoubleRow (K=256 in one pass),
    columns permuted to the (half, head, dlow) layout that makes the
    scores matmul a legit DoubleRow pairing [16 parts, 2, l].
    q gets +16*bq on the PSUM->SBUF copy (DVE); k needs no bias (the
    q.bk and bq.bk logit terms are constant per query -> softmax
    invariant, so bk is dropped exactly).
  - Scores tile [128,1024] = (2 lines, 4 heads): PSUM pre-loaded with
    rel_bias/ALPHA via an e5m2 hi+lo DoubleRow matmul against a
    stride-0 identity pair, scores accumulate on top, one Exp
    activation with scale=ALPHA drains it (no separate bias multiply).
  - V = x@(16*Wv) in 3 fp8 DR passes (hi*Whi + lo*Whi + hi*Wlo);
    per-head ones column holds 16.0 so the denominator needs no
    rescale. AV in bf16 (probs need > fp8 precision).
  - Y normalized by a broadcast divide (DVE), transposed to feature-
    major via blocked DMA-transpose (XBAR), projected with bf16
    matmuls, and written to a full [2ct,128h,128w] f32 SBUF staging
    buffer so the final DRAM writes are 8KB-contiguous per partition.
"""

import numpy as np
import ml_dtypes

import concourse.bacc as bacc
import concourse.bass as bass
import concourse.mybir as mybir
from concourse import tile
from concourse.bass import broadcast_tensor_aps
from concourse.bass_utils import run_bass_kernel_spmd

F8 = ml_dtypes.float8_e4m3fn
F85 = ml_dtypes.float8_e5m2
BF16 = ml_dtypes.bfloat16

B, T, C, H, W = 2, 4, 256, 128, 128
HEADS, DH = 8, 32
SCALE = DH ** (-0.5)
ALPHA = SCALE / 256.0
WBLK = 16
NBLK = W // WBLK  # 8
TOK = H * WBLK  # 2048 tokens per block

DT8 = mybir.dt.float8e4
DT85 = mybir.dt.float8e5
DTB = mybir.dt.bfloat16
DTF = mybir.dt.float32
AF = mybir.ActivationFunctionType
DR = mybir.MatmulPerfMode.DoubleRow
ADD = mybir.AluOpType.add
DIV = mybir.AluOpType.divide


DEBUG_TAPS = False


def build_program():
    nc = bacc.Bacc("TRN2")

    xh_d = nc.dram_tensor("xh", [128, NBLK * 2 * TOK], DT8, kind="ExternalInput")
    xl_d = nc.dram_tensor("xl", [128, NBLK * 2 * TOK], DT8, kind="ExternalInput")
    # fp8 consts: wq 512 | wk 512 | wvh 512 | wvl 512
    cb8 = nc.dram_tensor("cb8", [128, 2048], DT8, kind="ExternalInput")
    # e5m2 consts: ebt 2048 | id 128
    cb85 = nc.dram_tensor("cb85", [128, 2176], DT85, kind="ExternalInput")
    # bf16 consts: wo (ci, ct, 128)
    cb16 = nc.dram_tensor("cb16", [128, 512], DTB, kind="ExternalInput")
    # f32: bq16 (2 halves) | bout2 (2 ct)
    cf32 = nc.dram_tensor("cf32", [128, 4], DTF, kind="ExternalInput")
    out_bt = nc.dram_tensor("out_bt", [C, H, W], DTF, kind="ExternalOutput")

    with tile.TileContext(nc) as tc:
        with (
            tc.tile_pool(name="const", bufs=1) as cpool,
            tc.tile_pool(name="xs", bufs=2) as xpool,
            tc.tile_pool(name="qk", bufs=2) as qkpool,
            tc.tile_pool(name="vs", bufs=2) as vpool,
            tc.tile_pool(name="aw", bufs=3) as apool,
            tc.tile_pool(name="yn", bufs=2) as ynpool,
            tc.tile_pool(name="yt", bufs=1) as ytpool,
            tc.tile_pool(name="pss", bufs=3, space="PSUM") as ps_s,
            tc.tile_pool(name="psm", bufs=2, space="PSUM") as ps_m,
        ):
            cb8_t = cpool.tile([128, 2048], DT8, tag="cb8")
            nc.sync.dma_start(out=cb8_t[:], in_=cb8[:])
            cb85_t = cpool.tile([128, 2176], DT85, tag="cb85")
            nc.sync.dma_start(out=cb85_t[:], in_=cb85[:])
            cb16_t = cpool.tile([128, 512], DTB, tag="cb16")
            nc.sync.dma_start(out=cb16_t[:], in_=cb16[:])
            cf_t = cpool.tile([128, 4], DTF, tag="cf32")
            nc.sync.dma_start(out=cf_t[:], in_=cf32[:])

            wqv = cb8_t[:, 0:512].rearrange("p (a f) -> p a f", a=2)
            wkv = cb8_t[:, 512:1024].rearrange("p (a f) -> p a f", a=2)
            wvh = cb8_t[:, 1024:1536].rearrange("p (a f) -> p a f", a=2)
            wvl = cb8_t[:, 1536:2048].rearrange("p (a f) -> p a f", a=2)
            ebv = cb85_t[:, 0:2048].rearrange("p (i f) -> p i f", i=2)
            id3 = (
                cb85_t[:, 2048:2176]
                .rearrange("p (a f) -> p a f", a=1)
                .broadcast_to([128, 2, 128])
            )
            wov = cb16_t[:].rearrange("p (ci ct f) -> p ci ct f", ci=2, ct=2)

            # Output staging split into 5 w-slabs so slab DMAs can stream
            # during the loop without false deps: w-ranges
            # [0:32) [32:64) [64:96) [96:112) [112:128)
            SLABS = [(0, 32), (32, 64), (64, 96), (96, 112), (112, 128)]
            slab_of_wb = {0: 0, 1: 0, 2: 1, 3: 1, 4: 2, 5: 2, 6: 3, 7: 4}
            st_tiles = []
            for si, (w0, w1) in enumerate(SLABS):
                stt = cpool.tile([128, 2 * H * (w1 - w0)], DTF, tag=f"st{si}")
                st_tiles.append(
                    stt[:].rearrange("p (ct h w) -> p ct h w", ct=2, h=H)
                )

            tc.strict_bb_all_engine_barrier()

            x_tiles = {}

            def fetch_x(wb):
                xh_t = xpool.tile([128, 2 * TOK], DT8, tag="xh")
                nc.sync.dma_start(
                    out=xh_t[:], in_=xh_d[:, wb * 2 * TOK : (wb + 1) * 2 * TOK]
                )
                xl_t = xpool.tile([128, 2 * TOK], DT8, tag="xl")
                nc.sync.dma_start(
                    out=xl_t[:], in_=xl_d[:, wb * 2 * TOK : (wb + 1) * 2 * TOK]
                )
                x_tiles[wb] = (
                    xh_t[:].rearrange("p (a t) -> p a t", a=2),
                    xl_t[:].rearrange("p (a t) -> p a t", a=2),
                )

            qk_tiles = {}
            v_tiles = {}

            def stage_a_chunk(wb, i):
                # i in 0..7: (qk, half, ch)
                which, half, ch = i // 4, (i // 2) % 2, i % 2
                if i == 0:
                    q_sb = qkpool.tile([128, 2 * TOK], DT8, tag="q")
                    # k gets a 128-col zero strip (block 32) used as the dead
                    # half of the DoubleRow pair in the scores matmuls
                    k_sb = qkpool.tile([128, 2 * TOK + 128], DT8, tag="k")
                    nc.gpsimd.memset(k_sb[:, 2 * TOK :], 0.0)
                    qk_tiles[wb] = (q_sb, k_sb)
                q_sb, k_sb = qk_tiles[wb]
                xv = x_tiles[wb][0]
                w_l = (wqv, wkv)[which]
                dst = (q_sb, k_sb)[which]
                for sub in range(2):
                    ps = ps_m.tile([128, 512], DTF, tag="m")
                    for c4 in range(2):
                        t0 = ch * 1024 + sub * 512 + c4 * 256
                        nc.tensor.matmul(
                            ps[:, c4 * 256 : (c4 + 1) * 256],
                            lhsT=w_l[:, :, half * 128 : (half + 1) * 128],
                            rhs=xv[:, :, t0 : t0 + 256],
                            start=True,
                            stop=True,
                            perf_mode=DR,
                        )
                    t0 = half * TOK + ch * 1024 + sub * 512
                    dslice = dst[:, t0 : t0 + 512]
                    if which == 0:
                        nc.vector.tensor_scalar(
                            dslice, ps[:], cf_t[:, half : half + 1], None, ADD
                        )
                    else:
                        nc.gpsimd.tensor_copy(dslice, ps[:])

            def v_pair(wb, jp):
                # two lines jp*2, jp*2+1 -> v_sb slots
                if jp == 0:
                    v_sb = vpool.tile([128, WBLK * 264], DTB, tag="v")
                    v_tiles[wb] = v_sb
                    nc.vector.memset(
                        v_sb[:]
                        .rearrange("p (ln h d) -> p ln h d", ln=WBLK, h=HEADS)[
                            :, :, :, 32:33
                        ],
                        16.0,
                    )
                v_sb = v_tiles[wb]
                xv_h, xv_l = x_tiles[wb]
                ps = ps_m.tile([128, 512], DTF, tag="m")
                for j2 in range(2):
                    w = jp * 2 + j2
                    lx = slice(w * 128, (w + 1) * 128)
                    o = ps[:, j2 * 256 : (j2 + 1) * 256]
                    nc.tensor.matmul(
                        o, lhsT=xv_h[:, :, lx], rhs=wvh, start=True, stop=False,
                        perf_mode=DR,
                    )
                    nc.tensor.matmul(
                        o, lhsT=xv_l[:, :, lx], rhs=wvh, start=False, stop=False,
                        perf_mode=DR,
                    )
                    nc.tensor.matmul(
                        o, lhsT=xv_h[:, :, lx], rhs=wvl, start=False, stop=True,
                        perf_mode=DR,
                    )
                nc.gpsimd.tensor_copy(
                    v_sb[:]
                    .rearrange("p (ln h d) -> p ln h d", ln=WBLK, h=HEADS)[
                        :, jp * 2 : jp * 2 + 2, :, 0:32
                    ],
                    ps[:].rearrange("p (j2 h d) -> p j2 h d", j2=2, h=HEADS),
                )

            def s_tile(wb, jp, hg):
                # heads hg*4..hg*4+4 at partitions (h%4)*32, feature-half hg
                q_sb, k_sb = qk_tiles[wb]
                kv3 = k_sb[:].rearrange("p (blk c) -> p blk c", c=128)  # 33 blks
                ps = ps_s.tile([128, 1024], DTF, tag="s")
                ebr = ebv[:, :, hg * 512 : (hg + 1) * 512]
                for j2 in range(2):
                    nc.tensor.matmul(
                        ps[:, j2 * 512 : (j2 + 1) * 512],
                        lhsT=id3,
                        rhs=ebr,
                        start=True,
                        stop=False,
                        perf_mode=DR,
                        skip_group_check=True,
                    )
                    j = jp * 2 + j2
                    for hl in range(4):
                        hp = slice(hl * 32, (hl + 1) * 32)
                        blk = hg * 16 + j
                        lhsT = kv3[hp, blk : 33 : 32 - blk, :]  # [32,2,128]: k | 0
                        rhs = (
                            q_sb[hp, (hg * 16 + j) * 128 : (hg * 16 + j + 1) * 128]
                            .unsqueeze(1)
                            .to_broadcast([32, 2, 128])
                        )
                        nc.tensor.matmul(
                            ps[:, j2 * 512 + hl * 128 : j2 * 512 + (hl + 1) * 128],
                            lhsT=lhsT,
                            rhs=rhs,
                            start=False,
                            stop=True,
                            perf_mode=DR,
                            tile_position=(0, 0),
                            skip_group_check=True,
                        )
                aw = apool.tile([128, 1024], DTB, tag="aw")
                nc.scalar.activation(aw[:], ps[:], AF.Exp, scale=ALPHA)
                if DEBUG_TAPS and wb == 7 and jp == 0:
                    daw = nc.dram_tensor(
                        f"dbg_aw_{hg}", [128, 1024], DTB, kind="ExternalOutput"
                    )
                    nc.sync.dma_start(out=daw[:], in_=aw[:])
                return aw

            yn_tiles = {}

            def av_and_norm(wb, jp, aw0, aw1):
                v_sb = v_tiles[wb]
                g = jp // 4
                if jp % 4 == 0:
                    yn_t = ynpool.tile([128, 2048], DTB, tag="yn")
                    yn_tiles[(wb, g)] = yn_t
                yn_t = yn_tiles[(wb, g)]
                for j2 in range(2):
                    line = jp * 2 + j2
                    y_ps = ps_m.tile([128, 512], DTF, tag="m")
                    for h in range(HEADS):
                        aw = (aw0, aw1)[h // 4]
                        hl = h % 4
                        nc.tensor.matmul(
                            y_ps[:, h * 33 : h * 33 + 33],
                            lhsT=aw[:, j2 * 512 + hl * 128 : j2 * 512 + (hl + 1) * 128],
                            rhs=v_sb[:, line * 264 + h * 33 : line * 264 + h * 33 + 33],
                            start=True,
                            stop=True,
                        )
                    yv = y_ps[:, 0:264].rearrange("p (hh hl d) -> p hh hl d", hh=2, hl=4)
                    i0, i1 = broadcast_tensor_aps(yv[:, :, :, 0:32], yv[:, :, :, 32:33])
                    lg = (jp % 4) * 2 + j2
                    dst = (
                        yn_t[:]
                        .rearrange("p (hh lg c) -> p hh lg c", hh=2, lg=8)[
                            :, :, lg : lg + 1, :
                        ]
                        .rearrange("p hh lg (hl d) -> p hh (lg hl) d", hl=4)
                    )
                    nc.vector.tensor_tensor(
                        dst, i0, i1, DIV
                    )

            yt_tiles = {}

            def transpose_group(wb, g):
                if g == 0:
                    yt_t = ytpool.tile([128, 2 * 2048], DTB, tag="yt")
                    yt_tiles[wb] = yt_t
                # yt cols: (g 2, hh 2, line-in-group 8, l 128)
                yt_t = yt_tiles[wb]
                nc.sync.dma_start_transpose(
                    out=yt_t[:, g * 2048 : (g + 1) * 2048].rearrange(
                        "p (j l) -> p j l", l=128
                    ),
                    in_=yn_tiles[(wb, g)][:],
                )

            def oproj(wb, ch):
                yt_t = yt_tiles[wb]
                si = slab_of_wb[wb]
                wl = wb * WBLK - SLABS[si][0]
                for ct in range(2):
                    for sub in range(2):
                        ps = ps_m.tile([128, 512], DTF, tag="m")
                        for ci in range(2):
                            nc.tensor.matmul(
                                ps[:],
                                lhsT=wov[:, ci, ct, :],
                                rhs=yt_t[
                                    :,
                                    ch * 2048
                                    + ci * 1024
                                    + sub * 512 : ch * 2048
                                    + ci * 1024
                                    + (sub + 1) * 512,
                                ],
                                start=(ci == 0),
                                stop=(ci == 1),
                            )
                        w_lo = wl + ch * 8 + sub * 4
                        dst = st_tiles[si][:, ct, :, w_lo : w_lo + 4]
                        eng = nc.gpsimd if ct == 0 else nc.vector
                        eng.tensor_scalar(
                            dst.rearrange("p h w -> p w h"),
                            ps[:],
                            cf_t[:, 2 + ct : 3 + ct],
                            None,
                            ADD,
                        )

            def slab_dma(si, ct, hh):
                w0, w1 = SLABS[si]
                nc.sync.dma_start(
                    out=out_bt[ct * 128 : (ct + 1) * 128, :, w0:w1],
                    in_=st_tiles[si][:, ct, :, :],
                )

            # ---- prologue: block 0 inputs + stage A + V ----
            fetch_x(0)
            for i in range(8):
                stage_a_chunk(0, i)
            for jp in range(8):
                v_pair(0, jp)

            # ---- main loop: the tail of block wb (last AV, transpose g1,
            # oproj ch1, slab DMAs) is carried into block wb+1's interleave
            # slots so PE/Act never drain at block boundaries ----
            def block_tail_ops(wb, aws):
                yield lambda: av_and_norm(wb, 7, aws.pop((7, 0)), aws.pop((7, 1)))
                yield lambda: transpose_group(wb, 1)
                yield lambda: oproj(wb, 1)
                si = slab_of_wb[wb]
                if wb == max(k for k, v in slab_of_wb.items() if v == si):
                    yield lambda: [slab_dma(si, ct, 0) for ct in range(2)]

            carried = []
            for wb in range(NBLK):
                if wb + 1 < NBLK:
                    fetch_x(wb + 1)
                nxt = list(carried)
                carried = []
                if wb + 1 < NBLK:
                    nxt += [lambda i=i: stage_a_chunk(wb + 1, i) for i in range(8)] + [
                        lambda jp=jp: v_pair(wb + 1, jp) for jp in range(8)
                    ]
                aws = {}
                for jp in range(8):
                    aws[(jp, 0)] = s_tile(wb, jp, 0)
                    if jp >= 1:
                        av_and_norm(wb, jp - 1, aws.pop((jp - 1, 0)), aws.pop((jp - 1, 1)))
                    if nxt:
                        nxt.pop(0)()
                    aws[(jp, 1)] = s_tile(wb, jp, 1)
                    for _ in range(2):
                        if nxt:
                            nxt.pop(0)()
                    if jp == 5:
                        transpose_group(wb, 0)
                        oproj(wb, 0)
                while nxt:
                    nxt.pop(0)()
                carried = list(block_tail_ops(wb, aws))
            for op in carried:
                op()

            if DEBUG_TAPS:
                dq = nc.dram_tensor("dbg_q", [128, 2 * TOK], DT8, kind="ExternalOutput")
                dk = nc.dram_tensor("dbg_k", [128, 2 * TOK + 128], DT8, kind="ExternalOutput")
                dv = nc.dram_tensor("dbg_v", [128, WBLK * 264], DTB, kind="ExternalOutput")
                dyt = nc.dram_tensor("dbg_yt", [128, 2 * 2048], DTB, kind="ExternalOutput")
                nc.sync.dma_start(out=dq[:], in_=qk_tiles[7][0][:])
                nc.sync.dma_start(out=dk[:], in_=qk_tiles[7][1][:])
                nc.sync.dma_start(out=dv[:], in_=v_tiles[7][:])
                nc.sync.dma_start(out=dyt[:], in_=yt_tiles[7][:])
                dyn = nc.dram_tensor("dbg_yn", [128, 2048], DTB, kind="ExternalOutput")
                nc.sync.dma_start(out=dyn[:], in_=yn_tiles[(7, 0)][:])

    nc.compile()
    return nc


_NC = None


def _get_nc():
    global _NC
    if _NC is None:
        _NC = build_program()
    return _NC


def _prep_small(rel_bias, Wqkv, bqkv, Wout, bout):
    """Shared (per-core-identical) constant blobs."""
    # fp8 weight packs, x16 to stay in e4m3 normal range
    wq = (16.0 * Wqkv[:, 0:256]).reshape(2, 128, 256)
    wk = (16.0 * Wqkv[:, 256:512]).reshape(2, 128, 256)
    wv = (16.0 * Wqkv[:, 512:768]).reshape(2, 128, 256)
    # cb8 rows: partition p holds c = a*128+p at pair slot a
    wq8 = np.transpose(wq, (1, 0, 2)).reshape(128, 512).astype(F8)
    wk8 = np.transpose(wk, (1, 0, 2)).reshape(128, 512).astype(F8)
    wvf = np.transpose(wv, (1, 0, 2)).reshape(128, 512)
    wvh = wvf.astype(F8)
    wvl = (wvf - wvh.astype(np.float32)).astype(F8)
    cb8 = np.concatenate(
        [wq8.view(np.uint8), wk8.view(np.uint8), wvh.view(np.uint8), wvl.view(np.uint8)],
        axis=1,
    ).view(F8)

    # rel_bias^T / ALPHA as e5m2 hi+lo, layout [m, (h, l)]
    ebt = (rel_bias.transpose(0, 2, 1) / ALPHA).transpose(1, 0, 2).reshape(128, 1024)
    eb_hi = ebt.astype(F85)
    eb_lo = (ebt - eb_hi.astype(np.float32)).astype(F85)
    ident = np.eye(128, dtype=np.float32).astype(F85)
    cb85 = np.concatenate(
        [eb_hi.view(np.uint8), eb_lo.view(np.uint8), ident.view(np.uint8)], axis=1
    ).view(F85)

    wo = Wout.reshape(2, 128, 2, 128)  # (ci, p, ct, f)
    cb16 = np.transpose(wo, (1, 0, 2, 3)).reshape(128, 512).astype(BF16)

    bq16 = (16.0 * bqkv[0:256]).reshape(2, 128).T  # [128, 2]
    bout2 = (bout + bqkv[512:768] @ Wout).reshape(2, 128).T
    cf32 = np.concatenate([bq16, bout2], axis=1).astype(np.float32)
    return {
        "cb8": np.ascontiguousarray(cb8),
        "cb85": np.ascontiguousarray(cb85),
        "cb16": np.ascontiguousarray(cb16),
        "cf32": np.ascontiguousarray(cf32),
    }


def _prep_x(x):
    """Full-x fp8 hi/lo in the streamed DRAM layout, per (b,t)."""
    xf = np.asarray(x, np.float32)
    xh = xf.astype(F8)
    xl = (xf - xh.astype(np.float32)).astype(F8)

    def lay(a4):  # [256,128,128] -> [128, 8*2*16*128]
        v = a4.reshape(2, 128, H, W)  # (a, p, h, w)
        v = np.transpose(v, (1, 3, 0, 2))  # (p, w, a, h)
        v = v.reshape(128, NBLK, WBLK, 2, H)  # (p, wb, wl, a, h)
        v = np.transpose(v, (0, 1, 3, 2, 4))  # (p, wb, a, wl, h)
        return np.ascontiguousarray(v.reshape(128, NBLK * 2 * TOK))

    out = {}
    for b in range(B):
        for t in range(T):
            out[(b, t)] = (lay(xh[b, t]), lay(xl[b, t]))
    return out


def _core_in_maps(x, rel_bias, Wqkv, bqkv, Wout, bout):
    small = _prep_small(
        np.asarray(rel_bias, np.float32),
        np.asarray(Wqkv, np.float32),
        np.asarray(bqkv, np.float32),
        np.asarray(Wout, np.float32),
        np.asarray(bout, np.float32),
    )
    xs = _prep_x(x)
    maps = []
    for i in range(8):
        b, t = divmod(i, T)
        m = dict(small)
        m["xh"], m["xl"] = xs[(b, t)]
        maps.append(m)
    return maps


def _run(x, rel_bias, Wqkv, bqkv, Wout, bout, **spmd_kwargs):
    nc = _get_nc()
    in_maps = _core_in_maps(x, rel_bias, Wqkv, bqkv, Wout, bout)
    core_ids = list(range(8))
    res = run_bass_kernel_spmd(nc, in_maps, core_ids, **spmd_kwargs)
    out = np.empty((B, T, C, H, W), np.float32)
    for i in core_ids:
        b, t = divmod(i, T)
        out[b, t] = res.results[i]["out_bt"]
    return out, res


def kernel(x, rel_bias, Wqkv, bqkv, Wout, bout):
    out, _ = _run(x, rel_bias, Wqkv, bqkv, Wout, bout)
    return out


# revision 35
# speedup vs baseline: 1.2013x; 1.2013x over previous
"""AxialAttention (axis=height) Trainium2 Bass kernel, v2.

Problem: x [B=2,T=4,C=256,H=128,W=128] f32. Lines run along H; each core
owns one (b,t): 128 lines of length L=128, C=256 channels, 8 heads x 32.

Key structure (engine-balanced, fp8 DoubleRow heavy):
  - x streamed per w-block (16 lines) as fp8 hi+lo pair, host pre-laid-out
    so every DMA is 4KB-contiguous per partition.
  - Stage A: q,k = x_hi @ (16*Wqk) via fp8 DoubleRow (K=256 in one pass),
    columns permuted to the (half, head, dlow) layout that makes the
    scores matmul a legit DoubleRow pairing [16 parts, 2, l].
    q gets +16*bq on the PSUM->SBUF copy (DVE); k needs no bias (the
    q.bk and bq.bk logit terms are constant per query -> softmax
    invariant, so bk is dropped exactly).
  - Scores tile [128,1024] = (2 lines, 4 heads): PSUM pre-loaded with
    rel_bias/ALPHA via an e5m2 hi+lo DoubleRow matmul against a
    stride-0 identity pair, scores accumulate on top, one Exp
    activation with scale=ALPHA drains it (no separate bias multiply).
  - V = x@(16*Wv) in 3 fp8 DR passes (hi*Whi + lo*Whi + hi*Wlo);
    per-head ones column holds 16.0 so the denominator needs no
    rescale. AV in bf16 (probs need > fp8 precision).
  - Y normalized by a broadcast divide (DVE), transposed to feature-
    major via blocked DMA-transpose (XBAR), projected with bf16
    matmuls, and written to a full [2ct,128h,128w] f32 SBUF staging
    buffer so the final DRAM writes are 8KB-contiguous per partition.
"""

import numpy as np
import ml_dtypes

import concourse.bacc as bacc
import concourse.bass as bass
import concourse.mybir as mybir
from concourse import tile
from concourse.bass import broadcast_tensor_aps
from concourse.bass_utils import run_bass_kernel_spmd

F8 = ml_dtypes.float8_e4m3fn
F85 = ml_dtypes.float8_e5m2
BF16 = ml_dtypes.bfloat16

B, T, C, H, W = 2, 4, 256, 128, 128
HEADS, DH = 8, 32
SCALE = DH ** (-0.5)
ALPHA = SCALE / 256.0
WBLK = 16
NBLK = W // WBLK  # 8
TOK = H * WBLK  # 2048 tokens per block

DT8 = mybir.dt.float8e4
DT85 = mybir.dt.float8e5
DTB = mybir.dt.bfloat16
DTF = mybir.dt.float32
AF = mybir.ActivationFunctionType
DR = mybir.MatmulPerfMode.DoubleRow
ADD = mybir.AluOpType.add
DIV = mybir.AluOpType.divide


DEBUG_TAPS = False


def build_program():
    nc = bacc.Bacc("TRN2")

    xh_d = nc.dram_tensor("xh", [128, NBLK * 2 * TOK], DT8, kind="ExternalInput")
    xl_d = nc.dram_tensor("xl", [128, NBLK * 2 * TOK], DT8, kind="ExternalInput")
    # fp8 consts: wq 512 | wk 512 | wvh 512 | wvl 512
    cb8 = nc.dram_tensor("cb8", [128, 2048], DT8, kind="ExternalInput")
    # e5m2 consts: ebt 2048 | id 128
    cb85 = nc.dram_tensor("cb85", [128, 2176], DT85, kind="ExternalInput")
    # bf16 consts: wo (ci, ct, 128)
    cb16 = nc.dram_tensor("cb16", [128, 512], DTB, kind="ExternalInput")
    # f32: bq16 (2 halves) | bout2 (2 ct)
    cf32 = nc.dram_tensor("cf32", [128, 4], DTF, kind="ExternalInput")
    out_bt = nc.dram_tensor("out_bt", [C, H, W], DTF, kind="ExternalOutput")

    with tile.TileContext(nc) as tc:
        with (
            tc.tile_pool(name="const", bufs=1) as cpool,
            tc.tile_pool(name="xs", bufs=2) as xpool,
            tc.tile_pool(name="qk", bufs=2) as qkpool,
            tc.tile_pool(name="vs", bufs=2) as vpool,
            tc.tile_pool(name="aw", bufs=3) as apool,
            tc.tile_pool(name="yn", bufs=2) as ynpool,
            tc.tile_pool(name="yt", bufs=1) as ytpool,
            tc.tile_pool(name="pss", bufs=3, space="PSUM") as ps_s,
            tc.tile_pool(name="psm", bufs=2, space="PSUM") as ps_m,
        ):
            cb8_t = cpool.tile([128, 2048], DT8, tag="cb8")
            nc.sync.dma_start(out=cb8_t[:], in_=cb8[:])
            cb85_t = cpool.tile([128, 2176], DT85, tag="cb85")
            nc.sync.dma_start(out=cb85_t[:], in_=cb85[:])
            cb16_t = cpool.tile([128, 512], DTB, tag="cb16")
            nc.sync.dma_start(out=cb16_t[:], in_=cb16[:])
            cf_t = cpool.tile([128, 4], DTF, tag="cf32")
            nc.sync.dma_start(out=cf_t[:], in_=cf32[:])

            wqv = cb8_t[:, 0:512].rearrange("p (a f) -> p a f", a=2)
            wkv = cb8_t[:, 512:1024].rearrange("p (a f) -> p a f", a=2)
            wvh = cb8_t[:, 1024:1536].rearrange("p (a f) -> p a f", a=2)
            wvl = cb8_t[:, 1536:2048].rearrange("p (a f) -> p a f", a=2)
            ebv = cb85_t[:, 0:2048].rearrange("p (i f) -> p i f", i=2)
            id3 = (
                cb85_t[:, 2048:2176]
                .rearrange("p (a f) -> p a f", a=1)
                .broadcast_to([128, 2, 128])
            )
            wov = cb16_t[:].rearrange("p (ci ct f) -> p ci ct f", ci=2, ct=2)

            # Output staging split into 5 w-slabs so slab DMAs can stream
            # during the loop without false deps: w-ranges
            # [0:32) [32:64) [64:96) [96:112) [112:128)
            SLABS = [(0, 32), (32, 64), (64, 96), (96, 112), (112, 128)]
            slab_of_wb = {0: 0, 1: 0, 2: 1, 3: 1, 4: 2, 5: 2, 6: 3, 7: 4}
            st_tiles = []
            for si, (w0, w1) in enumerate(SLABS):
                stt = cpool.tile([128, 2 * H * (w1 - w0)], DTF, tag=f"st{si}")
                st_tiles.append(
                    stt[:].rearrange("p (ct h w) -> p ct h w", ct=2, h=H)
                )

            tc.strict_bb_all_engine_barrier()

            x_tiles = {}

            def fetch_x(wb):
                xh_t = xpool.tile([128, 2 * TOK], DT8, tag="xh")
                nc.sync.dma_start(
                    out=xh_t[:], in_=xh_d[:, wb * 2 * TOK : (wb + 1) * 2 * TOK]
                )
                xl_t = xpool.tile([128, 2 * TOK], DT8, tag="xl")
                nc.sync.dma_start(
                    out=xl_t[:], in_=xl_d[:, wb * 2 * TOK : (wb + 1) * 2 * TOK]
                )
                x_tiles[wb] = (
                    xh_t[:].rearrange("p (a t) -> p a t", a=2),
                    xl_t[:].rearrange("p (a t) -> p a t", a=2),
                )

            qk_tiles = {}
            v_tiles = {}

            def stage_a_chunk(wb, i):
                # i in 0..7: (qk, half, ch)
                which, half, ch = i // 4, (i // 2) % 2, i % 2
                if i == 0:
                    q_sb = qkpool.tile([128, 2 * TOK], DT8, tag="q")
                    # k gets a 128-col zero strip (block 32) used as the dead
                    # half of the DoubleRow pair in the scores matmuls
                    k_sb = qkpool.tile([128, 2 * TOK + 128], DT8, tag="k")
                    nc.gpsimd.memset(k_sb[:, 2 * TOK :], 0.0)
                    qk_tiles[wb] = (q_sb, k_sb)
                q_sb, k_sb = qk_tiles[wb]
                xv = x_tiles[wb][0]
                w_l = (wqv, wkv)[which]
                dst = (q_sb, k_sb)[which]
                for sub in range(2):
                    ps = ps_m.tile([128, 512], DTF, tag="m")
                    for c4 in range(2):
                        t0 = ch * 1024 + sub * 512 + c4 * 256
                        nc.tensor.matmul(
                            ps[:, c4 * 256 : (c4 + 1) * 256],
                            lhsT=w_l[:, :, half * 128 : (half + 1) * 128],
                            rhs=xv[:, :, t0 : t0 + 256],
                            start=True,
                            stop=True,
                            perf_mode=DR,
                        )
                    t0 = half * TOK + ch * 1024 + sub * 512
                    dslice = dst[:, t0 : t0 + 512]
                    if which == 0:
                        nc.vector.tensor_scalar(
                            dslice, ps[:], cf_t[:, half : half + 1], None, ADD
                        )
                    else:
                        nc.gpsimd.tensor_copy(dslice, ps[:])

            def v_pair(wb, jp):
                # two lines jp*2, jp*2+1 -> v_sb slots
                if jp == 0:
                    v_sb = vpool.tile([128, WBLK * 264], DTB, tag="v")
                    v_tiles[wb] = v_sb
                    nc.vector.memset(
                        v_sb[:]
                        .rearrange("p (ln h d) -> p ln h d", ln=WBLK, h=HEADS)[
                            :, :, :, 32:33
                        ],
                        16.0,
                    )
                v_sb = v_tiles[wb]
                xv_h, xv_l = x_tiles[wb]
                ps = ps_m.tile([128, 512], DTF, tag="m")
                for j2 in range(2):
                    w = jp * 2 + j2
                    lx = slice(w * 128, (w + 1) * 128)
                    o = ps[:, j2 * 256 : (j2 + 1) * 256]
                    nc.tensor.matmul(
                        o, lhsT=xv_h[:, :, lx], rhs=wvh, start=True, stop=False,
                        perf_mode=DR,
                    )
                    nc.tensor.matmul(
                        o, lhsT=xv_l[:, :, lx], rhs=wvh, start=False, stop=False,
                        perf_mode=DR,
                    )
                    nc.tensor.matmul(
                        o, lhsT=xv_h[:, :, lx], rhs=wvl, start=False, stop=True,
                        perf_mode=DR,
                    )
                nc.gpsimd.tensor_copy(
                    v_sb[:]
                    .rearrange("p (ln h d) -> p ln h d", ln=WBLK, h=HEADS)[
                        :, jp * 2 : jp * 2 + 2, :, 0:32
                    ],
                    ps[:].rearrange("p (j2 h d) -> p j2 h d", j2=2, h=HEADS),
                )

            def s_tile(wb, jp, hg):
                # heads hg*4..hg*4+4 at partitions (h%4)*32, feature-half hg
                q_sb, k_sb = qk_tiles[wb]
                kv3 = k_sb[:].rearrange("p (blk c) -> p blk c", c=128)  # 33 blks
                ps = ps_s.tile([128, 1024], DTF, tag="s")
                ebr = ebv[:, :, hg * 512 : (hg + 1) * 512]
                for j2 in range(2):
                    nc.tensor.matmul(
                        ps[:, j2 * 512 : (j2 + 1) * 512],
                        lhsT=id3,
                        rhs=ebr,
                        start=True,
                        stop=False,
                        perf_mode=DR,
                        skip_group_check=True,
                    )
                    j = jp * 2 + j2
                    for hl in range(4):
                        hp = slice(hl * 32, (hl + 1) * 32)
                        blk = hg * 16 + j
                        lhsT = kv3[hp, blk : 33 : 32 - blk, :]  # [32,2,128]: k | 0
                        rhs = (
                            q_sb[hp, (hg * 16 + j) * 128 : (hg * 16 + j + 1) * 128]
                            .unsqueeze(1)
                            .to_broadcast([32, 2, 128])
                        )
                        nc.tensor.matmul(
                            ps[:, j2 * 512 + hl * 128 : j2 * 512 + (hl + 1) * 128],
                            lhsT=lhsT,
                            rhs=rhs,
                            start=False,
                            stop=True,
                            perf_mode=DR,
                            tile_position=(0, 0),
                            skip_group_check=True,
                        )
                aw = apool.tile([128, 1024], DTB, tag="aw")
                nc.scalar.activation(aw[:], ps[:], AF.Exp, scale=ALPHA)
                if DEBUG_TAPS and wb == 7 and jp == 0:
                    daw = nc.dram_tensor(
                        f"dbg_aw_{hg}", [128, 1024], DTB, kind="ExternalOutput"
                    )
                    nc.sync.dma_start(out=daw[:], in_=aw[:])
                return aw

            yn_tiles = {}

            def av_and_norm(wb, jp, aw0, aw1):
                v_sb = v_tiles[wb]
                y_ps = ps_s.tile([128, 1024], DTF, tag="s")
                for j2 in range(2):
                    line = jp * 2 + j2
                    for h in range(HEADS):
                        aw = (aw0, aw1)[h // 4]
                        hl = h % 4
                        nc.tensor.matmul(
                            y_ps[:, j2 * 512 + h * 33 : j2 * 512 + h * 33 + 33],
                            lhsT=aw[:, j2 * 512 + hl * 128 : j2 * 512 + (hl + 1) * 128],
                            rhs=v_sb[:, line * 264 + h * 33 : line * 264 + h * 33 + 33],
                            start=True,
                            stop=True,
                        )
                g = jp // 4
                if jp % 4 == 0:
                    yn_t = ynpool.tile([128, 2048], DTB, tag="yn")
                    yn_tiles[(wb, g)] = yn_t
                yn_t = yn_tiles[(wb, g)]
                yv = (
                    y_ps[:]
                    .rearrange("p (j2 f) -> p j2 f", j2=2)[:, :, 0:264]
                    .rearrange("p j2 (hh hl d) -> p hh j2 hl d", hh=2, hl=4)
                )
                i0, i1 = broadcast_tensor_aps(yv[:, :, :, :, 0:32], yv[:, :, :, :, 32:33])
                lg = (jp % 4) * 2
                dst = (
                    yn_t[:]
                    .rearrange("p (hh lg c) -> p hh lg c", hh=2, lg=8)[
                        :, :, lg : lg + 2, :
                    ]
                    .rearrange("p hh lg (hl d) -> p hh lg hl d", hl=4)
                )
                nc.vector.tensor_tensor(dst, i0, i1, DIV)

            yt_tiles = {}

            def transpose_group(wb, g):
                if g == 0:
                    yt_t = ytpool.tile([128, 2 * 2048], DTB, tag="yt")
                    yt_tiles[wb] = yt_t
                # yt cols: (g 2, hh 2, line-in-group 8, l 128)
                yt_t = yt_tiles[wb]
                nc.sync.dma_start_transpose(
                    out=yt_t[:, g * 2048 : (g + 1) * 2048].rearrange(
                        "p (j l) -> p j l", l=128
                    ),
                    in_=yn_tiles[(wb, g)][:],
                )

            def oproj(wb, ch):
                yt_t = yt_tiles[wb]
                si = slab_of_wb[wb]
                wl = wb * WBLK - SLABS[si][0]
                for ct in range(2):
                    for sub in range(2):
                        ps = ps_m.tile([128, 512], DTF, tag="m")
                        for ci in range(2):
                            nc.tensor.matmul(
                                ps[:],
                                lhsT=wov[:, ci, ct, :],
                                rhs=yt_t[
                                    :,
                                    ch * 2048
                                    + ci * 1024
                                    + sub * 512 : ch * 2048
                                    + ci * 1024
                                    + (sub + 1) * 512,
                                ],
                                start=(ci == 0),
                                stop=(ci == 1),
                            )
                        w_lo = wl + ch * 8 + sub * 4
                        dst = st_tiles[si][:, ct, :, w_lo : w_lo + 4]
                        eng = nc.gpsimd if ct == 0 else nc.vector
                        eng.tensor_scalar(
                            dst.rearrange("p h w -> p w h"),
                            ps[:],
                            cf_t[:, 2 + ct : 3 + ct],
                            None,
                            ADD,
                        )

            def slab_dma(si, ct, hh):
                w0, w1 = SLABS[si]
                nc.sync.dma_start(
                    out=out_bt[ct * 128 : (ct + 1) * 128, :, w0:w1],
                    in_=st_tiles[si][:, ct, :, :],
                )

            # ---- prologue: block 0 inputs + stage A + V ----
            fetch_x(0)
            for i in range(8):
                stage_a_chunk(0, i)
            for jp in range(8):
                v_pair(0, jp)

            # ---- main loop: the tail of block wb (last AV, transpose g1,
            # oproj ch1, slab DMAs) is carried into block wb+1's interleave
            # slots so PE/Act never drain at block boundaries ----
            def block_tail_ops(wb, aws):
                yield lambda: av_and_norm(wb, 7, aws.pop((7, 0)), aws.pop((7, 1)))
                yield lambda: transpose_group(wb, 1)
                yield lambda: oproj(wb, 1)
                si = slab_of_wb[wb]
                if wb == max(k for k, v in slab_of_wb.items() if v == si):
                    yield lambda: [slab_dma(si, ct, 0) for ct in range(2)]

            carried = []
            for wb in range(NBLK):
                if wb + 1 < NBLK:
                    fetch_x(wb + 1)
                nxt = list(carried)
                carried = []
                if wb + 1 < NBLK:
                    nxt += [lambda i=i: stage_a_chunk(wb + 1, i) for i in range(8)] + [
                        lambda jp=jp: v_pair(wb + 1, jp) for jp in range(8)
                    ]
                aws = {}
                for jp in range(8):
                    aws[(jp, 0)] = s_tile(wb, jp, 0)
                    if jp >= 1:
                        av_and_norm(wb, jp - 1, aws.pop((jp - 1, 0)), aws.pop((jp - 1, 1)))
                    if nxt:
                        nxt.pop(0)()
                    aws[(jp, 1)] = s_tile(wb, jp, 1)
                    for _ in range(2):
                        if nxt:
                            nxt.pop(0)()
                    if jp == 5:
                        transpose_group(wb, 0)
                        oproj(wb, 0)
                while nxt:
                    nxt.pop(0)()
                carried = list(block_tail_ops(wb, aws))
            for op in carried:
                op()

            if DEBUG_TAPS:
                dq = nc.dram_tensor("dbg_q", [128, 2 * TOK], DT8, kind="ExternalOutput")
                dk = nc.dram_tensor("dbg_k", [128, 2 * TOK + 128], DT8, kind="ExternalOutput")
                dv = nc.dram_tensor("dbg_v", [128, WBLK * 264], DTB, kind="ExternalOutput")
                dyt = nc.dram_tensor("dbg_yt", [128, 2 * 2048], DTB, kind="ExternalOutput")
                nc.sync.dma_start(out=dq[:], in_=qk_tiles[7][0][:])
                nc.sync.dma_start(out=dk[:], in_=qk_tiles[7][1][:])
                nc.sync.dma_start(out=dv[:], in_=v_tiles[7][:])
                nc.sync.dma_start(out=dyt[:], in_=yt_tiles[7][:])
                dyn = nc.dram_tensor("dbg_yn", [128, 2048], DTB, kind="ExternalOutput")
                nc.sync.dma_start(out=dyn[:], in_=yn_tiles[(7, 0)][:])

    nc.compile()
    return nc


_NC = None


def _get_nc():
    global _NC
    if _NC is None:
        _NC = build_program()
    return _NC


def _prep_small(rel_bias, Wqkv, bqkv, Wout, bout):
    """Shared (per-core-identical) constant blobs."""
    # fp8 weight packs, x16 to stay in e4m3 normal range
    wq = (16.0 * Wqkv[:, 0:256]).reshape(2, 128, 256)
    wk = (16.0 * Wqkv[:, 256:512]).reshape(2, 128, 256)
    wv = (16.0 * Wqkv[:, 512:768]).reshape(2, 128, 256)
    # cb8 rows: partition p holds c = a*128+p at pair slot a
    wq8 = np.transpose(wq, (1, 0, 2)).reshape(128, 512).astype(F8)
    wk8 = np.transpose(wk, (1, 0, 2)).reshape(128, 512).astype(F8)
    wvf = np.transpose(wv, (1, 0, 2)).reshape(128, 512)
    wvh = wvf.astype(F8)
    wvl = (wvf - wvh.astype(np.float32)).astype(F8)
    cb8 = np.concatenate(
        [wq8.view(np.uint8), wk8.view(np.uint8), wvh.view(np.uint8), wvl.view(np.uint8)],
        axis=1,
    ).view(F8)

    # rel_bias^T / ALPHA as e5m2 hi+lo, layout [m, (h, l)]
    ebt = (rel_bias.transpose(0, 2, 1) / ALPHA).transpose(1, 0, 2).reshape(128, 1024)
    eb_hi = ebt.astype(F85)
    eb_lo = (ebt - eb_hi.astype(np.float32)).astype(F85)
    ident = np.eye(128, dtype=np.float32).astype(F85)
    cb85 = np.concatenate(
        [eb_hi.view(np.uint8), eb_lo.view(np.uint8), ident.view(np.uint8)], axis=1
    ).view(F85)

    wo = Wout.reshape(2, 128, 2, 128)  # (ci, p, ct, f)
    cb16 = np.transpose(wo, (1, 0, 2, 3)).reshape(128, 512).astype(BF16)

    bq16 = (16.0 * bqkv[0:256]).reshape(2, 128).T  # [128, 2]
    bout2 = (bout + bqkv[512:768] @ Wout).reshape(2, 128).T
    cf32 = np.concatenate([bq16, bout2], axis=1).astype(np.float32)
    return {
        "cb8": np.ascontiguousarray(cb8),
        "cb85": np.ascontiguousarray(cb85),
        "cb16": np.ascontiguousarray(cb16),
        "cf32": np.ascontiguousarray(cf32),
    }


def _prep_x(x):
    """Full-x fp8 hi/lo in the streamed DRAM layout, per (b,t)."""
    xf = np.asarray(x, np.float32)
    xh = xf.astype(F8)
    xl = (xf - xh.astype(np.float32)).astype(F8)

    def lay(a4):  # [256,128,128] -> [128, 8*2*16*128]
        v = a4.reshape(2, 128, H, W)  # (a, p, h, w)
        v = np.transpose(v, (1, 3, 0, 2))  # (p, w, a, h)
        v = v.reshape(128, NBLK, WBLK, 2, H)  # (p, wb, wl, a, h)
        v = np.transpose(v, (0, 1, 3, 2, 4))  # (p, wb, a, wl, h)
        return np.ascontiguousarray(v.reshape(128, NBLK * 2 * TOK))

    out = {}
    for b in range(B):
        for t in range(T):
            out[(b, t)] = (lay(xh[b, t]), lay(xl[b, t]))
    return out


def _core_in_maps(x, rel_bias, Wqkv, bqkv, Wout, bout):
    small = _prep_small(
        np.asarray(rel_bias, np.float32),
        np.asarray(Wqkv, np.float32),
        np.asarray(bqkv, np.float32),
        np.asarray(Wout, np.float32),
        np.asarray(bout, np.float32),
    )
    xs = _prep_x(x)
    maps = []
    for i in range(8):
        b, t = divmod(i, T)
        m = dict(small)
        m["xh"], m["xl"] = xs[(b, t)]
        maps.append(m)
    return maps


def _run(x, rel_bias, Wqkv, bqkv, Wout, bout, **spmd_kwargs):
    nc = _get_nc()
    in_maps = _core_in_maps(x, rel_bias, Wqkv, bqkv, Wout, bout)
    core_ids = list(range(8))
    res = run_bass_kernel_spmd(nc, in_maps, core_ids, **spmd_kwargs)
    out = np.empty((B, T, C, H, W), np.float32)
    for i in core_ids:
        b, t = divmod(i, T)
        out[b, t] = res.results[i]["out_bt"]
    return out, res


def kernel(x, rel_bias, Wqkv, bqkv, Wout, bout):
    out, _ = _run(x, rel_bias, Wqkv, bqkv, Wout, bout)
    return out


# revision 36
# speedup vs baseline: 1.2143x; 1.0108x over previous
"""AxialAttention (axis=height) Trainium2 Bass kernel, v2.

Problem: x [B=2,T=4,C=256,H=128,W=128] f32. Lines run along H; each core
owns one (b,t): 128 lines of length L=128, C=256 channels, 8 heads x 32.

Key structure (engine-balanced, fp8 DoubleRow heavy):
  - x streamed per w-block (16 lines) as fp8 hi+lo pair, host pre-laid-out
    so every DMA is 4KB-contiguous per partition.
  - Stage A: q,k = x_hi @ (16*Wqk) via fp8 DoubleRow (K=256 in one pass),
    columns permuted to the (half, head, dlow) layout that makes the
    scores matmul a legit DoubleRow pairing [16 parts, 2, l].
    q gets +16*bq on the PSUM->SBUF copy (DVE); k needs no bias (the
    q.bk and bq.bk logit terms are constant per query -> softmax
    invariant, so bk is dropped exactly).
  - Scores tile [128,1024] = (2 lines, 4 heads): PSUM pre-loaded with
    rel_bias/ALPHA via an e5m2 hi+lo DoubleRow matmul against a
    stride-0 identity pair, scores accumulate on top, one Exp
    activation with scale=ALPHA drains it (no separate bias multiply).
  - V = x@(16*Wv) in 3 fp8 DR passes (hi*Whi + lo*Whi + hi*Wlo);
    per-head ones column holds 16.0 so the denominator needs no
    rescale. AV in bf16 (probs need > fp8 precision).
  - Y normalized by a broadcast divide (DVE), transposed to feature-
    major via blocked DMA-transpose (XBAR), projected with bf16
    matmuls, and written to a full [2ct,128h,128w] f32 SBUF staging
    buffer so the final DRAM writes are 8KB-contiguous per partition.
"""

import numpy as np
import ml_dtypes

import concourse.bacc as bacc
import concourse.bass as bass
import concourse.mybir as mybir
from concourse import tile
from concourse.bass import broadcast_tensor_aps
from concourse.bass_utils import run_bass_kernel_spmd

F8 = ml_dtypes.float8_e4m3fn
F85 = ml_dtypes.float8_e5m2
BF16 = ml_dtypes.bfloat16

B, T, C, H, W = 2, 4, 256, 128, 128
HEADS, DH = 8, 32
SCALE = DH ** (-0.5)
ALPHA = SCALE / 256.0
WBLK = 16
NBLK = W // WBLK  # 8
TOK = H * WBLK  # 2048 tokens per block

DT8 = mybir.dt.float8e4
DT85 = mybir.dt.float8e5
DTB = mybir.dt.bfloat16
DTF = mybir.dt.float32
AF = mybir.ActivationFunctionType
DR = mybir.MatmulPerfMode.DoubleRow
ADD = mybir.AluOpType.add
DIV = mybir.AluOpType.divide


DEBUG_TAPS = False


def build_program():
    nc = bacc.Bacc("TRN2")

    xh_d = nc.dram_tensor("xh", [128, NBLK * 2 * TOK], DT8, kind="ExternalInput")
    xl_d = nc.dram_tensor("xl", [128, NBLK * 2 * TOK], DT8, kind="ExternalInput")
    # fp8 consts: wq 512 | wk 512 | wvh 512 | wvl 512
    cb8 = nc.dram_tensor("cb8", [128, 2048], DT8, kind="ExternalInput")
    # e5m2 consts: ebt 2048 | id 128
    cb85 = nc.dram_tensor("cb85", [128, 2176], DT85, kind="ExternalInput")
    # bf16 consts: wo (ci, ct, 128)
    cb16 = nc.dram_tensor("cb16", [128, 512], DTB, kind="ExternalInput")
    # f32: bq16 (2 halves) | bout2 (2 ct)
    cf32 = nc.dram_tensor("cf32", [128, 4], DTF, kind="ExternalInput")
    out_bt = nc.dram_tensor("out_bt", [C, H, W], DTF, kind="ExternalOutput")

    with tile.TileContext(nc) as tc:
        with (
            tc.tile_pool(name="const", bufs=1) as cpool,
            tc.tile_pool(name="xs", bufs=2) as xpool,
            tc.tile_pool(name="qk", bufs=2) as qkpool,
            tc.tile_pool(name="vs", bufs=2) as vpool,
            tc.tile_pool(name="aw", bufs=3) as apool,
            tc.tile_pool(name="yn", bufs=2) as ynpool,
            tc.tile_pool(name="yt", bufs=1) as ytpool,
            tc.tile_pool(name="pss", bufs=3, space="PSUM") as ps_s,
            tc.tile_pool(name="psm", bufs=2, space="PSUM") as ps_m,
        ):
            cb8_t = cpool.tile([128, 2048], DT8, tag="cb8")
            nc.sync.dma_start(out=cb8_t[:], in_=cb8[:])
            cb85_t = cpool.tile([128, 2176], DT85, tag="cb85")
            nc.sync.dma_start(out=cb85_t[:], in_=cb85[:])
            cb16_t = cpool.tile([128, 512], DTB, tag="cb16")
            nc.sync.dma_start(out=cb16_t[:], in_=cb16[:])
            cf_t = cpool.tile([128, 4], DTF, tag="cf32")
            nc.sync.dma_start(out=cf_t[:], in_=cf32[:])

            wqv = cb8_t[:, 0:512].rearrange("p (a f) -> p a f", a=2)
            wkv = cb8_t[:, 512:1024].rearrange("p (a f) -> p a f", a=2)
            wvh = cb8_t[:, 1024:1536].rearrange("p (a f) -> p a f", a=2)
            wvl = cb8_t[:, 1536:2048].rearrange("p (a f) -> p a f", a=2)
            ebv = cb85_t[:, 0:2048].rearrange("p (i f) -> p i f", i=2)
            id3 = (
                cb85_t[:, 2048:2176]
                .rearrange("p (a f) -> p a f", a=1)
                .broadcast_to([128, 2, 128])
            )
            wov = cb16_t[:].rearrange("p (ci ct f) -> p ci ct f", ci=2, ct=2)

            # Output staging split into 5 w-slabs so slab DMAs can stream
            # during the loop without false deps: w-ranges
            # [0:32) [32:64) [64:96) [96:112) [112:128)
            SLABS = [(0, 32), (32, 64), (64, 96), (96, 112), (112, 128)]
            slab_of_wb = {0: 0, 1: 0, 2: 1, 3: 1, 4: 2, 5: 2, 6: 3, 7: 4}
            st_tiles = []
            for si, (w0, w1) in enumerate(SLABS):
                stt = cpool.tile([128, 2 * H * (w1 - w0)], DTF, tag=f"st{si}")
                st_tiles.append(
                    stt[:].rearrange("p (ct h w) -> p ct h w", ct=2, h=H)
                )

            tc.strict_bb_all_engine_barrier()

            x_tiles = {}

            def fetch_x(wb):
                xh_t = xpool.tile([128, 2 * TOK], DT8, tag="xh")
                nc.sync.dma_start(
                    out=xh_t[:], in_=xh_d[:, wb * 2 * TOK : (wb + 1) * 2 * TOK]
                )
                xl_t = xpool.tile([128, 2 * TOK], DT8, tag="xl")
                nc.sync.dma_start(
                    out=xl_t[:], in_=xl_d[:, wb * 2 * TOK : (wb + 1) * 2 * TOK]
                )
                x_tiles[wb] = (
                    xh_t[:].rearrange("p (a t) -> p a t", a=2),
                    xl_t[:].rearrange("p (a t) -> p a t", a=2),
                )

            qk_tiles = {}
            v_tiles = {}

            def stage_a_chunk(wb, i):
                # i in 0..7: (qk, half, ch)
                which, half, ch = i // 4, (i // 2) % 2, i % 2
                if i == 0:
                    q_sb = qkpool.tile([128, 2 * TOK], DT8, tag="q")
                    # k gets a 128-col zero strip (block 32) used as the dead
                    # half of the DoubleRow pair in the scores matmuls
                    k_sb = qkpool.tile([128, 2 * TOK + 128], DT8, tag="k")
                    nc.gpsimd.memset(k_sb[:, 2 * TOK :], 0.0)
                    qk_tiles[wb] = (q_sb, k_sb)
                q_sb, k_sb = qk_tiles[wb]
                xv = x_tiles[wb][0]
                w_l = (wqv, wkv)[which]
                dst = (q_sb, k_sb)[which]
                for sub in range(2):
                    ps = ps_m.tile([128, 512], DTF, tag="m")
                    for c4 in range(2):
                        t0 = ch * 1024 + sub * 512 + c4 * 256
                        nc.tensor.matmul(
                            ps[:, c4 * 256 : (c4 + 1) * 256],
                            lhsT=w_l[:, :, half * 128 : (half + 1) * 128],
                            rhs=xv[:, :, t0 : t0 + 256],
                            start=True,
                            stop=True,
                            perf_mode=DR,
                        )
                    t0 = half * TOK + ch * 1024 + sub * 512
                    dslice = dst[:, t0 : t0 + 512]
                    if which == 0:
                        nc.vector.tensor_scalar(
                            dslice, ps[:], cf_t[:, half : half + 1], None, ADD
                        )
                    else:
                        nc.gpsimd.tensor_copy(dslice, ps[:])

            def v_pair(wb, jp):
                # two lines jp*2, jp*2+1 -> v_sb slots
                if jp == 0:
                    v_sb = vpool.tile([128, WBLK * 264], DTB, tag="v")
                    v_tiles[wb] = v_sb
                    nc.vector.memset(
                        v_sb[:]
                        .rearrange("p (ln h d) -> p ln h d", ln=WBLK, h=HEADS)[
                            :, :, :, 32:33
                        ],
                        16.0,
                    )
                v_sb = v_tiles[wb]
                xv_h, xv_l = x_tiles[wb]
                ps = ps_m.tile([128, 512], DTF, tag="m")
                for j2 in range(2):
                    w = jp * 2 + j2
                    lx = slice(w * 128, (w + 1) * 128)
                    o = ps[:, j2 * 256 : (j2 + 1) * 256]
                    nc.tensor.matmul(
                        o, lhsT=xv_h[:, :, lx], rhs=wvh, start=True, stop=False,
                        perf_mode=DR,
                    )
                    nc.tensor.matmul(
                        o, lhsT=xv_l[:, :, lx], rhs=wvh, start=False, stop=False,
                        perf_mode=DR,
                    )
                    nc.tensor.matmul(
                        o, lhsT=xv_h[:, :, lx], rhs=wvl, start=False, stop=True,
                        perf_mode=DR,
                    )
                nc.gpsimd.tensor_copy(
                    v_sb[:]
                    .rearrange("p (ln h d) -> p ln h d", ln=WBLK, h=HEADS)[
                        :, jp * 2 : jp * 2 + 2, :, 0:32
                    ],
                    ps[:].rearrange("p (j2 h d) -> p j2 h d", j2=2, h=HEADS),
                )

            def s_tile(wb, jp, hg):
                # heads hg*4..hg*4+4 at partitions (h%4)*32, feature-half hg
                q_sb, k_sb = qk_tiles[wb]
                kv3 = k_sb[:].rearrange("p (blk c) -> p blk c", c=128)  # 33 blks
                ps = ps_s.tile([128, 1024], DTF, tag="s")
                ebr = ebv[:, :, hg * 512 : (hg + 1) * 512]
                for j2 in range(2):
                    nc.tensor.matmul(
                        ps[:, j2 * 512 : (j2 + 1) * 512],
                        lhsT=id3,
                        rhs=ebr,
                        start=True,
                        stop=False,
                        perf_mode=DR,
                        skip_group_check=True,
                    )
                    j = jp * 2 + j2
                    for hl in range(4):
                        hp = slice(hl * 32, (hl + 1) * 32)
                        blk = hg * 16 + j
                        lhsT = kv3[hp, blk : 33 : 32 - blk, :]  # [32,2,128]: k | 0
                        rhs = (
                            q_sb[hp, (hg * 16 + j) * 128 : (hg * 16 + j + 1) * 128]
                            .unsqueeze(1)
                            .to_broadcast([32, 2, 128])
                        )
                        nc.tensor.matmul(
                            ps[:, j2 * 512 + hl * 128 : j2 * 512 + (hl + 1) * 128],
                            lhsT=lhsT,
                            rhs=rhs,
                            start=False,
                            stop=True,
                            perf_mode=DR,
                            tile_position=(0, 0),
                            skip_group_check=True,
                        )
                aw = apool.tile([128, 1024], DTB, tag="aw")
                nc.scalar.activation(aw[:], ps[:], AF.Exp, scale=ALPHA)
                if DEBUG_TAPS and wb == 7 and jp == 0:
                    daw = nc.dram_tensor(
                        f"dbg_aw_{hg}", [128, 1024], DTB, kind="ExternalOutput"
                    )
                    nc.sync.dma_start(out=daw[:], in_=aw[:])
                return aw

            yn_tiles = {}

            def av_and_norm(wb, jp, aw0, aw1):
                v_sb = v_tiles[wb]
                y_ps = ps_s.tile([128, 1024], DTF, tag="s")
                for j2 in range(2):
                    line = jp * 2 + j2
                    for h in range(HEADS):
                        aw = (aw0, aw1)[h // 4]
                        hl = h % 4
                        nc.tensor.matmul(
                            y_ps[:, j2 * 512 + h * 33 : j2 * 512 + h * 33 + 33],
                            lhsT=aw[:, j2 * 512 + hl * 128 : j2 * 512 + (hl + 1) * 128],
                            rhs=v_sb[:, line * 264 + h * 33 : line * 264 + h * 33 + 33],
                            start=True,
                            stop=True,
                        )
                g = jp // 4
                if jp % 4 == 0:
                    yn_t = ynpool.tile([128, 2048], DTB, tag="yn")
                    yn_tiles[(wb, g)] = yn_t
                yn_t = yn_tiles[(wb, g)]
                yv = (
                    y_ps[:]
                    .rearrange("p (j2 f) -> p j2 f", j2=2)[:, :, 0:264]
                    .rearrange("p j2 (hh hl d) -> p hh j2 hl d", hh=2, hl=4)
                )
                i0, i1 = broadcast_tensor_aps(yv[:, :, :, :, 0:32], yv[:, :, :, :, 32:33])
                lg = (jp % 4) * 2
                dst = (
                    yn_t[:]
                    .rearrange("p (hh lg c) -> p hh lg c", hh=2, lg=8)[
                        :, :, lg : lg + 2, :
                    ]
                    .rearrange("p hh lg (hl d) -> p hh lg hl d", hl=4)
                )
                nc.vector.tensor_tensor(dst, i0, i1, DIV)

            yt_tiles = {}

            def transpose_group(wb, g):
                if g == 0:
                    yt_t = ytpool.tile([128, 2 * 2048], DTB, tag="yt")
                    yt_tiles[wb] = yt_t
                # yt cols: (g 2, hh 2, line-in-group 8, l 128)
                yt_t = yt_tiles[wb]
                nc.sync.dma_start_transpose(
                    out=yt_t[:, g * 2048 : (g + 1) * 2048].rearrange(
                        "p (j l) -> p j l", l=128
                    ),
                    in_=yn_tiles[(wb, g)][:],
                )

            def oproj(wb, ch):
                yt_t = yt_tiles[wb]
                si = slab_of_wb[wb]
                wl = wb * WBLK - SLABS[si][0]
                for ct in range(2):
                    for sub in range(2):
                        ps = ps_m.tile([128, 512], DTF, tag="m")
                        for ci in range(2):
                            nc.tensor.matmul(
                                ps[:],
                                lhsT=wov[:, ci, ct, :],
                                rhs=yt_t[
                                    :,
                                    ch * 2048
                                    + ci * 1024
                                    + sub * 512 : ch * 2048
                                    + ci * 1024
                                    + (sub + 1) * 512,
                                ],
                                start=(ci == 0),
                                stop=(ci == 1),
                            )
                        w_lo = wl + ch * 8 + sub * 4
                        dst = st_tiles[si][:, ct, :, w_lo : w_lo + 4]
                        eng = nc.gpsimd if ct == 0 else nc.vector
                        eng.tensor_scalar(
                            dst.rearrange("p h w -> p w h"),
                            ps[:],
                            cf_t[:, 2 + ct : 3 + ct],
                            None,
                            ADD,
                        )

            def slab_dma(si, ct, hh):
                w0, w1 = SLABS[si]
                nc.sync.dma_start(
                    out=out_bt[ct * 128 : (ct + 1) * 128, :, w0:w1],
                    in_=st_tiles[si][:, ct, :, :],
                )

            # ---- prologue: block 0 inputs + stage A + V ----
            fetch_x(0)
            for i in range(8):
                stage_a_chunk(0, i)
            for jp in range(8):
                v_pair(0, jp)

            # ---- main loop: the tail of block wb (last AV, transpose g1,
            # oproj ch1, slab DMAs) is carried into block wb+1's interleave
            # slots so PE/Act never drain at block boundaries ----
            def block_tail_ops(wb, aws):
                yield lambda: av_and_norm(wb, 7, aws.pop((7, 0)), aws.pop((7, 1)))
                yield lambda: transpose_group(wb, 1)
                yield lambda: oproj(wb, 1)
                si = slab_of_wb[wb]
                if wb == max(k for k, v in slab_of_wb.items() if v == si):
                    yield lambda: [slab_dma(si, ct, 0) for ct in range(2)]

            carried = []
            for wb in range(NBLK):
                if wb + 1 < NBLK:
                    fetch_x(wb + 1)
                nxt = list(carried)
                carried = []
                if wb + 1 < NBLK:
                    nxt += [lambda i=i: stage_a_chunk(wb + 1, i) for i in range(8)] + [
                        lambda jp=jp: v_pair(wb + 1, jp) for jp in range(8)
                    ]
                aws = {}
                for jp in range(8):
                    aws[(jp, 0)] = s_tile(wb, jp, 0)
                    if nxt:
                        nxt.pop(0)()
                    aws[(jp, 1)] = s_tile(wb, jp, 1)
                    if jp >= 1:
                        av_and_norm(wb, jp - 1, aws.pop((jp - 1, 0)), aws.pop((jp - 1, 1)))
                    for _ in range(2):
                        if nxt:
                            nxt.pop(0)()
                    if jp == 5:
                        transpose_group(wb, 0)
                        oproj(wb, 0)
                while nxt:
                    nxt.pop(0)()
                carried = list(block_tail_ops(wb, aws))
            for op in carried:
                op()

            if DEBUG_TAPS:
                dq = nc.dram_tensor("dbg_q", [128, 2 * TOK], DT8, kind="ExternalOutput")
                dk = nc.dram_tensor("dbg_k", [128, 2 * TOK + 128], DT8, kind="ExternalOutput")
                dv = nc.dram_tensor("dbg_v", [128, WBLK * 264], DTB, kind="ExternalOutput")
                dyt = nc.dram_tensor("dbg_yt", [128, 2 * 2048], DTB, kind="ExternalOutput")
                nc.sync.dma_start(out=dq[:], in_=qk_tiles[7][0][:])
                nc.sync.dma_start(out=dk[:], in_=qk_tiles[7][1][:])
                nc.sync.dma_start(out=dv[:], in_=v_tiles[7][:])
                nc.sync.dma_start(out=dyt[:], in_=yt_tiles[7][:])
                dyn = nc.dram_tensor("dbg_yn", [128, 2048], DTB, kind="ExternalOutput")
                nc.sync.dma_start(out=dyn[:], in_=yn_tiles[(7, 0)][:])

    nc.compile()
    return nc


_NC = None


def _get_nc():
    global _NC
    if _NC is None:
        _NC = build_program()
    return _NC


def _prep_small(rel_bias, Wqkv, bqkv, Wout, bout):
    """Shared (per-core-identical) constant blobs."""
    # fp8 weight packs, x16 to stay in e4m3 normal range
    wq = (16.0 * Wqkv[:, 0:256]).reshape(2, 128, 256)
    wk = (16.0 * Wqkv[:, 256:512]).reshape(2, 128, 256)
    wv = (16.0 * Wqkv[:, 512:768]).reshape(2, 128, 256)
    # cb8 rows: partition p holds c = a*128+p at pair slot a
    wq8 = np.transpose(wq, (1, 0, 2)).reshape(128, 512).astype(F8)
    wk8 = np.transpose(wk, (1, 0, 2)).reshape(128, 512).astype(F8)
    wvf = np.transpose(wv, (1, 0, 2)).reshape(128, 512)
    wvh = wvf.astype(F8)
    wvl = (wvf - wvh.astype(np.float32)).astype(F8)
    cb8 = np.concatenate(
        [wq8.view(np.uint8), wk8.view(np.uint8), wvh.view(np.uint8), wvl.view(np.uint8)],
        axis=1,
    ).view(F8)

    # rel_bias^T / ALPHA as e5m2 hi+lo, layout [m, (h, l)]
    ebt = (rel_bias.transpose(0, 2, 1) / ALPHA).transpose(1, 0, 2).reshape(128, 1024)
    eb_hi = ebt.astype(F85)
    eb_lo = (ebt - eb_hi.astype(np.float32)).astype(F85)
    ident = np.eye(128, dtype=np.float32).astype(F85)
    cb85 = np.concatenate(
        [eb_hi.view(np.uint8), eb_lo.view(np.uint8), ident.view(np.uint8)], axis=1
    ).view(F85)

    wo = Wout.reshape(2, 128, 2, 128)  # (ci, p, ct, f)
    cb16 = np.transpose(wo, (1, 0, 2, 3)).reshape(128, 512).astype(BF16)

    bq16 = (16.0 * bqkv[0:256]).reshape(2, 128).T  # [128, 2]
    bout2 = (bout + bqkv[512:768] @ Wout).reshape(2, 128).T
    cf32 = np.concatenate([bq16, bout2], axis=1).astype(np.float32)
    return {
        "cb8": np.ascontiguousarray(cb8),
        "cb85": np.ascontiguousarray(cb85),
        "cb16": np.ascontiguousarray(cb16),
        "cf32": np.ascontiguousarray(cf32),
    }


def _prep_x(x):
    """Full-x fp8 hi/lo in the streamed DRAM layout, per (b,t)."""
    xf = np.asarray(x, np.float32)
    xh = xf.astype(F8)
    xl = (xf - xh.astype(np.float32)).astype(F8)

    def lay(a4):  # [256,128,128] -> [128, 8*2*16*128]
        v = a4.reshape(2, 128, H, W)  # (a, p, h, w)
        v = np.transpose(v, (1, 3, 0, 2))  # (p, w, a, h)
        v = v.reshape(128, NBLK, WBLK, 2, H)  # (p, wb, wl, a, h)
        v = np.transpose(v, (0, 1, 3, 2, 4))  # (p, wb, a, wl, h)
        return np.ascontiguousarray(v.reshape(128, NBLK * 2 * TOK))

    out = {}
    for b in range(B):
        for t in range(T):
            out[(b, t)] = (lay(xh[b, t]), lay(xl[b, t]))
    return out


def _core_in_maps(x, rel_bias, Wqkv, bqkv, Wout, bout):
    small = _prep_small(
        np.asarray(rel_bias, np.float32),
        np.asarray(Wqkv, np.float32),
        np.asarray(bqkv, np.float32),
        np.asarray(Wout, np.float32),
        np.asarray(bout, np.float32),
    )
    xs = _prep_x(x)
    maps = []
    for i in range(8):
        b, t = divmod(i, T)
        m = dict(small)
        m["xh"], m["xl"] = xs[(b, t)]
        maps.append(m)
    return maps


def _run(x, rel_bias, Wqkv, bqkv, Wout, bout, **spmd_kwargs):
    nc = _get_nc()
    in_maps = _core_in_maps(x, rel_bias, Wqkv, bqkv, Wout, bout)
    core_ids = list(range(8))
    res = run_bass_kernel_spmd(nc, in_maps, core_ids, **spmd_kwargs)
    out = np.empty((B, T, C, H, W), np.float32)
    for i in core_ids:
        b, t = divmod(i, T)
        out[b, t] = res.results[i]["out_bt"]
    return out, res


def kernel(x, rel_bias, Wqkv, bqkv, Wout, bout):
    out, _ = _run(x, rel_bias, Wqkv, bqkv, Wout, bout)
    return out


# revision 38
# speedup vs baseline: 1.2204x; 1.0050x over previous
"""AxialAttention (axis=height) Trainium2 Bass kernel, v2.

Problem: x [B=2,T=4,C=256,H=128,W=128] f32. Lines run along H; each core
owns one (b,t): 128 lines of length L=128, C=256 channels, 8 heads x 32.

Key structure (engine-balanced, fp8 DoubleRow heavy):
  - x streamed per w-block (16 lines) as fp8 hi+lo pair, host pre-laid-out
    so every DMA is 4KB-contiguous per partition.
  - Stage A: q,k = x_hi @ (16*Wqk) via fp8 DoubleRow (K=256 in one pass),
    columns permuted to the (half, head, dlow) layout that makes the
    scores matmul a legit DoubleRow pairing [16 parts, 2, l].
    q gets +16*bq on the PSUM->SBUF copy (DVE); k needs no bias (the
    q.bk and bq.bk logit terms are constant per query -> softmax
    invariant, so bk is dropped exactly).
  - Scores tile [128,1024] = (2 lines, 4 heads): PSUM pre-loaded with
    rel_bias/ALPHA via an e5m2 hi+lo DoubleRow matmul against a
    stride-0 identity pair, scores accumulate on top, one Exp
    activation with scale=ALPHA drains it (no separate bias multiply).
  - V = x@(16*Wv) in 3 fp8 DR passes (hi*Whi + lo*Whi + hi*Wlo);
    per-head ones column holds 16.0 so the denominator needs no
    rescale. AV in bf16 (probs need > fp8 precision).
  - Y normalized by a broadcast divide (DVE), transposed to feature-
    major via blocked DMA-transpose (XBAR), projected with bf16
    matmuls, and written to a full [2ct,128h,128w] f32 SBUF staging
    buffer so the final DRAM writes are 8KB-contiguous per partition.
"""

import numpy as np
import ml_dtypes

import concourse.bacc as bacc
import concourse.bass as bass
import concourse.mybir as mybir
from concourse import tile
from concourse.bass import broadcast_tensor_aps
from concourse.bass_utils import run_bass_kernel_spmd

F8 = ml_dtypes.float8_e4m3fn
F85 = ml_dtypes.float8_e5m2
BF16 = ml_dtypes.bfloat16

B, T, C, H, W = 2, 4, 256, 128, 128
HEADS, DH = 8, 32
SCALE = DH ** (-0.5)
ALPHA = SCALE / 256.0
WBLK = 16
NBLK = W // WBLK  # 8
TOK = H * WBLK  # 2048 tokens per block

DT8 = mybir.dt.float8e4
DT85 = mybir.dt.float8e5
DTB = mybir.dt.bfloat16
DTF = mybir.dt.float32
AF = mybir.ActivationFunctionType
DR = mybir.MatmulPerfMode.DoubleRow
ADD = mybir.AluOpType.add
DIV = mybir.AluOpType.divide


DEBUG_TAPS = False


def build_program():
    nc = bacc.Bacc("TRN2")

    xh_d = nc.dram_tensor("xh", [128, NBLK * 2 * TOK], DT8, kind="ExternalInput")
    xl_d = nc.dram_tensor("xl", [128, NBLK * 2 * TOK], DT8, kind="ExternalInput")
    # fp8 consts: wq 512 | wk 512 | wvh 512 | wvl 512
    cb8 = nc.dram_tensor("cb8", [128, 2048], DT8, kind="ExternalInput")
    # e5m2 consts: ebt 2048 | id 128
    cb85 = nc.dram_tensor("cb85", [128, 2176], DT85, kind="ExternalInput")
    # bf16 consts: wo (ci, ct, 128)
    cb16 = nc.dram_tensor("cb16", [128, 512], DTB, kind="ExternalInput")
    # f32: bq16 (2 halves) | bout2 (2 ct)
    cf32 = nc.dram_tensor("cf32", [128, 4], DTF, kind="ExternalInput")
    out_bt = nc.dram_tensor("out_bt", [C, H, W], DTF, kind="ExternalOutput")

    with tile.TileContext(nc) as tc:
        with (
            tc.tile_pool(name="const", bufs=1) as cpool,
            tc.tile_pool(name="xs", bufs=2) as xpool,
            tc.tile_pool(name="qk", bufs=2) as qkpool,
            tc.tile_pool(name="vs", bufs=2) as vpool,
            tc.tile_pool(name="aw", bufs=4) as apool,
            tc.tile_pool(name="yn", bufs=2) as ynpool,
            tc.tile_pool(name="yt", bufs=1) as ytpool,
            tc.tile_pool(name="pss", bufs=3, space="PSUM") as ps_s,
            tc.tile_pool(name="psm", bufs=2, space="PSUM") as ps_m,
        ):
            cb8_t = cpool.tile([128, 2048], DT8, tag="cb8")
            nc.sync.dma_start(out=cb8_t[:], in_=cb8[:])
            cb85_t = cpool.tile([128, 2176], DT85, tag="cb85")
            nc.sync.dma_start(out=cb85_t[:], in_=cb85[:])
            cb16_t = cpool.tile([128, 512], DTB, tag="cb16")
            nc.sync.dma_start(out=cb16_t[:], in_=cb16[:])
            cf_t = cpool.tile([128, 4], DTF, tag="cf32")
            nc.sync.dma_start(out=cf_t[:], in_=cf32[:])

            wqv = cb8_t[:, 0:512].rearrange("p (a f) -> p a f", a=2)
            wkv = cb8_t[:, 512:1024].rearrange("p (a f) -> p a f", a=2)
            wvh = cb8_t[:, 1024:1536].rearrange("p (a f) -> p a f", a=2)
            wvl = cb8_t[:, 1536:2048].rearrange("p (a f) -> p a f", a=2)
            ebv = cb85_t[:, 0:2048].rearrange("p (i f) -> p i f", i=2)
            id3 = (
                cb85_t[:, 2048:2176]
                .rearrange("p (a f) -> p a f", a=1)
                .broadcast_to([128, 2, 128])
            )
            wov = cb16_t[:].rearrange("p (ci ct f) -> p ci ct f", ci=2, ct=2)

            # Output staging split into 5 w-slabs so slab DMAs can stream
            # during the loop without false deps: w-ranges
            # [0:32) [32:64) [64:96) [96:112) [112:128)
            SLABS = [(0, 32), (32, 64), (64, 96), (96, 112), (112, 128)]
            slab_of_wb = {0: 0, 1: 0, 2: 1, 3: 1, 4: 2, 5: 2, 6: 3, 7: 4}
            st_tiles = []
            for si, (w0, w1) in enumerate(SLABS):
                stt = cpool.tile([128, 2 * H * (w1 - w0)], DTF, tag=f"st{si}")
                st_tiles.append(
                    stt[:].rearrange("p (ct h w) -> p ct h w", ct=2, h=H)
                )

            tc.strict_bb_all_engine_barrier()

            x_tiles = {}

            def fetch_x(wb):
                xh_t = xpool.tile([128, 2 * TOK], DT8, tag="xh")
                nc.sync.dma_start(
                    out=xh_t[:], in_=xh_d[:, wb * 2 * TOK : (wb + 1) * 2 * TOK]
                )
                xl_t = xpool.tile([128, 2 * TOK], DT8, tag="xl")
                nc.sync.dma_start(
                    out=xl_t[:], in_=xl_d[:, wb * 2 * TOK : (wb + 1) * 2 * TOK]
                )
                x_tiles[wb] = (
                    xh_t[:].rearrange("p (a t) -> p a t", a=2),
                    xl_t[:].rearrange("p (a t) -> p a t", a=2),
                )

            qk_tiles = {}
            v_tiles = {}

            def stage_a_chunk(wb, i):
                # i in 0..7: (qk, half, ch)
                which, half, ch = i // 4, (i // 2) % 2, i % 2
                if i == 0:
                    q_sb = qkpool.tile([128, 2 * TOK], DT8, tag="q")
                    # k gets a 128-col zero strip (block 32) used as the dead
                    # half of the DoubleRow pair in the scores matmuls
                    k_sb = qkpool.tile([128, 2 * TOK + 128], DT8, tag="k")
                    nc.gpsimd.memset(k_sb[:, 2 * TOK :], 0.0)
                    qk_tiles[wb] = (q_sb, k_sb)
                q_sb, k_sb = qk_tiles[wb]
                xv = x_tiles[wb][0]
                w_l = (wqv, wkv)[which]
                dst = (q_sb, k_sb)[which]
                for sub in range(2):
                    ps = ps_m.tile([128, 512], DTF, tag="m")
                    for c4 in range(2):
                        t0 = ch * 1024 + sub * 512 + c4 * 256
                        nc.tensor.matmul(
                            ps[:, c4 * 256 : (c4 + 1) * 256],
                            lhsT=w_l[:, :, half * 128 : (half + 1) * 128],
                            rhs=xv[:, :, t0 : t0 + 256],
                            start=True,
                            stop=True,
                            perf_mode=DR,
                        )
                    t0 = half * TOK + ch * 1024 + sub * 512
                    dslice = dst[:, t0 : t0 + 512]
                    if which == 0:
                        nc.vector.tensor_scalar(
                            dslice, ps[:], cf_t[:, half : half + 1], None, ADD
                        )
                    else:
                        nc.gpsimd.tensor_copy(dslice, ps[:])

            def v_pair(wb, jp):
                # two lines jp*2, jp*2+1 -> v_sb slots
                if jp == 0:
                    v_sb = vpool.tile([128, WBLK * 264], DTB, tag="v")
                    v_tiles[wb] = v_sb
                    nc.vector.memset(
                        v_sb[:]
                        .rearrange("p (ln h d) -> p ln h d", ln=WBLK, h=HEADS)[
                            :, :, :, 32:33
                        ],
                        16.0,
                    )
                v_sb = v_tiles[wb]
                xv_h, xv_l = x_tiles[wb]
                ps = ps_m.tile([128, 512], DTF, tag="m")
                for j2 in range(2):
                    w = jp * 2 + j2
                    lx = slice(w * 128, (w + 1) * 128)
                    o = ps[:, j2 * 256 : (j2 + 1) * 256]
                    nc.tensor.matmul(
                        o, lhsT=xv_h[:, :, lx], rhs=wvh, start=True, stop=False,
                        perf_mode=DR,
                    )
                    nc.tensor.matmul(
                        o, lhsT=xv_l[:, :, lx], rhs=wvh, start=False, stop=False,
                        perf_mode=DR,
                    )
                    nc.tensor.matmul(
                        o, lhsT=xv_h[:, :, lx], rhs=wvl, start=False, stop=True,
                        perf_mode=DR,
                    )
                nc.gpsimd.tensor_copy(
                    v_sb[:]
                    .rearrange("p (ln h d) -> p ln h d", ln=WBLK, h=HEADS)[
                        :, jp * 2 : jp * 2 + 2, :, 0:32
                    ],
                    ps[:].rearrange("p (j2 h d) -> p j2 h d", j2=2, h=HEADS),
                )

            def s_tile(wb, jp, hg):
                # heads hg*4..hg*4+4 at partitions (h%4)*32, feature-half hg
                q_sb, k_sb = qk_tiles[wb]
                kv3 = k_sb[:].rearrange("p (blk c) -> p blk c", c=128)  # 33 blks
                ps = ps_s.tile([128, 1024], DTF, tag="s")
                ebr = ebv[:, :, hg * 512 : (hg + 1) * 512]
                for j2 in range(2):
                    nc.tensor.matmul(
                        ps[:, j2 * 512 : (j2 + 1) * 512],
                        lhsT=id3,
                        rhs=ebr,
                        start=True,
                        stop=False,
                        perf_mode=DR,
                        skip_group_check=True,
                    )
                    j = jp * 2 + j2
                    for hl in range(4):
                        hp = slice(hl * 32, (hl + 1) * 32)
                        blk = hg * 16 + j
                        lhsT = kv3[hp, blk : 33 : 32 - blk, :]  # [32,2,128]: k | 0
                        rhs = (
                            q_sb[hp, (hg * 16 + j) * 128 : (hg * 16 + j + 1) * 128]
                            .unsqueeze(1)
                            .to_broadcast([32, 2, 128])
                        )
                        nc.tensor.matmul(
                            ps[:, j2 * 512 + hl * 128 : j2 * 512 + (hl + 1) * 128],
                            lhsT=lhsT,
                            rhs=rhs,
                            start=False,
                            stop=True,
                            perf_mode=DR,
                            tile_position=(0, 0),
                            skip_group_check=True,
                        )
                aw = apool.tile([128, 1024], DTB, tag="aw")
                nc.scalar.activation(aw[:], ps[:], AF.Exp, scale=ALPHA)
                if DEBUG_TAPS and wb == 7 and jp == 0:
                    daw = nc.dram_tensor(
                        f"dbg_aw_{hg}", [128, 1024], DTB, kind="ExternalOutput"
                    )
                    nc.sync.dma_start(out=daw[:], in_=aw[:])
                return aw

            yn_tiles = {}

            def av_and_norm(wb, jp, aw0, aw1):
                v_sb = v_tiles[wb]
                y_ps = ps_s.tile([128, 1024], DTF, tag="s")
                for j2 in range(2):
                    line = jp * 2 + j2
                    for h in range(HEADS):
                        aw = (aw0, aw1)[h // 4]
                        hl = h % 4
                        nc.tensor.matmul(
                            y_ps[:, j2 * 512 + h * 33 : j2 * 512 + h * 33 + 33],
                            lhsT=aw[:, j2 * 512 + hl * 128 : j2 * 512 + (hl + 1) * 128],
                            rhs=v_sb[:, line * 264 + h * 33 : line * 264 + h * 33 + 33],
                            start=True,
                            stop=True,
                        )
                g = jp // 4
                if jp % 4 == 0:
                    yn_t = ynpool.tile([128, 2048], DTB, tag="yn")
                    yn_tiles[(wb, g)] = yn_t
                yn_t = yn_tiles[(wb, g)]
                yv = (
                    y_ps[:]
                    .rearrange("p (j2 f) -> p j2 f", j2=2)[:, :, 0:264]
                    .rearrange("p j2 (hh hl d) -> p hh j2 hl d", hh=2, hl=4)
                )
                i0, i1 = broadcast_tensor_aps(yv[:, :, :, :, 0:32], yv[:, :, :, :, 32:33])
                lg = (jp % 4) * 2
                dst = (
                    yn_t[:]
                    .rearrange("p (hh lg c) -> p hh lg c", hh=2, lg=8)[
                        :, :, lg : lg + 2, :
                    ]
                    .rearrange("p hh lg (hl d) -> p hh lg hl d", hl=4)
                )
                nc.vector.tensor_tensor(dst, i0, i1, DIV)

            yt_tiles = {}

            def transpose_group(wb, g):
                if g == 0:
                    yt_t = ytpool.tile([128, 2 * 2048], DTB, tag="yt")
                    yt_tiles[wb] = yt_t
                # yt cols: (g 2, hh 2, line-in-group 8, l 128)
                yt_t = yt_tiles[wb]
                nc.sync.dma_start_transpose(
                    out=yt_t[:, g * 2048 : (g + 1) * 2048].rearrange(
                        "p (j l) -> p j l", l=128
                    ),
                    in_=yn_tiles[(wb, g)][:],
                )

            def oproj(wb, ch):
                yt_t = yt_tiles[wb]
                si = slab_of_wb[wb]
                wl = wb * WBLK - SLABS[si][0]
                for ct in range(2):
                    for sub in range(2):
                        ps = ps_m.tile([128, 512], DTF, tag="m")
                        for ci in range(2):
                            nc.tensor.matmul(
                                ps[:],
                                lhsT=wov[:, ci, ct, :],
                                rhs=yt_t[
                                    :,
                                    ch * 2048
                                    + ci * 1024
                                    + sub * 512 : ch * 2048
                                    + ci * 1024
                                    + (sub + 1) * 512,
                                ],
                                start=(ci == 0),
                                stop=(ci == 1),
                            )
                        w_lo = wl + ch * 8 + sub * 4
                        dst = st_tiles[si][:, ct, :, w_lo : w_lo + 4]
                        eng = nc.gpsimd if ct == 0 else nc.vector
                        eng.tensor_scalar(
                            dst.rearrange("p h w -> p w h"),
                            ps[:],
                            cf_t[:, 2 + ct : 3 + ct],
                            None,
                            ADD,
                        )

            def slab_dma(si, ct, hh):
                w0, w1 = SLABS[si]
                nc.sync.dma_start(
                    out=out_bt[ct * 128 : (ct + 1) * 128, :, w0:w1],
                    in_=st_tiles[si][:, ct, :, :],
                )

            # ---- prologue: block 0 inputs + stage A + V ----
            fetch_x(0)
            for i in range(8):
                stage_a_chunk(0, i)
            for jp in range(8):
                v_pair(0, jp)

            # ---- main loop: the tail of block wb (last AV, transpose g1,
            # oproj ch1, slab DMAs) is carried into block wb+1's interleave
            # slots so PE/Act never drain at block boundaries ----
            def block_tail_ops(wb, aws):
                yield lambda: av_and_norm(wb, 7, aws.pop((7, 0)), aws.pop((7, 1)))
                yield lambda: transpose_group(wb, 1)
                yield lambda: oproj(wb, 1)
                si = slab_of_wb[wb]
                if wb == max(k for k, v in slab_of_wb.items() if v == si):
                    yield lambda: [slab_dma(si, ct, 0) for ct in range(2)]

            carried = []
            for wb in range(NBLK):
                if wb + 1 < NBLK:
                    fetch_x(wb + 1)
                nxt = list(carried)
                carried = []
                if wb + 1 < NBLK:
                    nxt += [lambda i=i: stage_a_chunk(wb + 1, i) for i in range(8)] + [
                        lambda jp=jp: v_pair(wb + 1, jp) for jp in range(8)
                    ]
                aws = {}
                for jp in range(8):
                    aws[(jp, 0)] = s_tile(wb, jp, 0)
                    if nxt:
                        nxt.pop(0)()
                    aws[(jp, 1)] = s_tile(wb, jp, 1)
                    if jp >= 1:
                        av_and_norm(wb, jp - 1, aws.pop((jp - 1, 0)), aws.pop((jp - 1, 1)))
                    for _ in range(2):
                        if nxt:
                            nxt.pop(0)()
                    if jp == 5:
                        transpose_group(wb, 0)
                        oproj(wb, 0)
                while nxt:
                    nxt.pop(0)()
                carried = list(block_tail_ops(wb, aws))
            for op in carried:
                op()

            if DEBUG_TAPS:
                dq = nc.dram_tensor("dbg_q", [128, 2 * TOK], DT8, kind="ExternalOutput")
                dk = nc.dram_tensor("dbg_k", [128, 2 * TOK + 128], DT8, kind="ExternalOutput")
                dv = nc.dram_tensor("dbg_v", [128, WBLK * 264], DTB, kind="ExternalOutput")
                dyt = nc.dram_tensor("dbg_yt", [128, 2 * 2048], DTB, kind="ExternalOutput")
                nc.sync.dma_start(out=dq[:], in_=qk_tiles[7][0][:])
                nc.sync.dma_start(out=dk[:], in_=qk_tiles[7][1][:])
                nc.sync.dma_start(out=dv[:], in_=v_tiles[7][:])
                nc.sync.dma_start(out=dyt[:], in_=yt_tiles[7][:])
                dyn = nc.dram_tensor("dbg_yn", [128, 2048], DTB, kind="ExternalOutput")
                nc.sync.dma_start(out=dyn[:], in_=yn_tiles[(7, 0)][:])

    nc.compile()
    return nc


_NC = None


def _get_nc():
    global _NC
    if _NC is None:
        _NC = build_program()
    return _NC


def _prep_small(rel_bias, Wqkv, bqkv, Wout, bout):
    """Shared (per-core-identical) constant blobs."""
    # fp8 weight packs, x16 to stay in e4m3 normal range
    wq = (16.0 * Wqkv[:, 0:256]).reshape(2, 128, 256)
    wk = (16.0 * Wqkv[:, 256:512]).reshape(2, 128, 256)
    wv = (16.0 * Wqkv[:, 512:768]).reshape(2, 128, 256)
    # cb8 rows: partition p holds c = a*128+p at pair slot a
    wq8 = np.transpose(wq, (1, 0, 2)).reshape(128, 512).astype(F8)
    wk8 = np.transpose(wk, (1, 0, 2)).reshape(128, 512).astype(F8)
    wvf = np.transpose(wv, (1, 0, 2)).reshape(128, 512)
    wvh = wvf.astype(F8)
    wvl = (wvf - wvh.astype(np.float32)).astype(F8)
    cb8 = np.concatenate(
        [wq8.view(np.uint8), wk8.view(np.uint8), wvh.view(np.uint8), wvl.view(np.uint8)],
        axis=1,
    ).view(F8)

    # rel_bias^T / ALPHA as e5m2 hi+lo, layout [m, (h, l)]
    ebt = (rel_bias.transpose(0, 2, 1) / ALPHA).transpose(1, 0, 2).reshape(128, 1024)
    eb_hi = ebt.astype(F85)
    eb_lo = (ebt - eb_hi.astype(np.float32)).astype(F85)
    ident = np.eye(128, dtype=np.float32).astype(F85)
    cb85 = np.concatenate(
        [eb_hi.view(np.uint8), eb_lo.view(np.uint8), ident.view(np.uint8)], axis=1
    ).view(F85)

    wo = Wout.reshape(2, 128, 2, 128)  # (ci, p, ct, f)
    cb16 = np.transpose(wo, (1, 0, 2, 3)).reshape(128, 512).astype(BF16)

    bq16 = (16.0 * bqkv[0:256]).reshape(2, 128).T  # [128, 2]
    bout2 = (bout + bqkv[512:768] @ Wout).reshape(2, 128).T
    cf32 = np.concatenate([bq16, bout2], axis=1).astype(np.float32)
    return {
        "cb8": np.ascontiguousarray(cb8),
        "cb85": np.ascontiguousarray(cb85),
        "cb16": np.ascontiguousarray(cb16),
        "cf32": np.ascontiguousarray(cf32),
    }


def _prep_x(x):
    """Full-x fp8 hi/lo in the streamed DRAM layout, per (b,t)."""
    xf = np.asarray(x, np.float32)
    xh = xf.astype(F8)
    xl = (xf - xh.astype(np.float32)).astype(F8)

    def lay(a4):  # [256,128,128] -> [128, 8*2*16*128]
        v = a4.reshape(2, 128, H, W)  # (a, p, h, w)
        v = np.transpose(v, (1, 3, 0, 2))  # (p, w, a, h)
        v = v.reshape(128, NBLK, WBLK, 2, H)  # (p, wb, wl, a, h)
        v = np.transpose(v, (0, 1, 3, 2, 4))  # (p, wb, a, wl, h)
        return np.ascontiguousarray(v.reshape(128, NBLK * 2 * TOK))

    out = {}
    for b in range(B):
        for t in range(T):
            out[(b, t)] = (lay(xh[b, t]), lay(xl[b, t]))
    return out


def _core_in_maps(x, rel_bias, Wqkv, bqkv, Wout, bout):
    small = _prep_small(
        np.asarray(rel_bias, np.float32),
        np.asarray(Wqkv, np.float32),
        np.asarray(bqkv, np.float32),
        np.asarray(Wout, np.float32),
        np.asarray(bout, np.float32),
    )
    xs = _prep_x(x)
    maps = []
    for i in range(8):
        b, t = divmod(i, T)
        m = dict(small)
        m["xh"], m["xl"] = xs[(b, t)]
        maps.append(m)
    return maps


def _run(x, rel_bias, Wqkv, bqkv, Wout, bout, **spmd_kwargs):
    nc = _get_nc()
    in_maps = _core_in_maps(x, rel_bias, Wqkv, bqkv, Wout, bout)
    core_ids = list(range(8))
    res = run_bass_kernel_spmd(nc, in_maps, core_ids, **spmd_kwargs)
    out = np.empty((B, T, C, H, W), np.float32)
    for i in core_ids:
        b, t = divmod(i, T)
        out[b, t] = res.results[i]["out_bt"]
    return out, res


def kernel(x, rel_bias, Wqkv, bqkv, Wout, bout):
    out, _ = _run(x, rel_bias, Wqkv, bqkv, Wout, bout)
    return out
